# revision 7
# baseline (speedup 1.0000x reference)
"""AttnPointNetConv on 8 TRN2 NeuronCores.

Strategy (edge-parallel, dst-sharded):
  - Host: sort edges by (dst-core, src-chunk, dst-group). Core c owns nodes
    [c*12544, (c+1)*12544). src space split into 4 chunks of 25600 so
    dma_gather's int16 indices can address the x-table chunk.
  - Pass 1: per-edge gather of x[src]; PE accumulates the Gram matrix
    G = sum(m~ m~^T) of the augmented message m~ = [1, pos_j-pos_i, x_j].
    Host derives BN1 stats from G (linear algebra) and folds BN1 into the
    MLP weight W' (affine fold), incl. exact removal of padding edges.
  - Pass 2: recompute m~, h = silu(m~ @ W'), PE accumulates h~-Gram for the
    gate BN stats; saves h (bf16) and gate_pre (f32) to DRAM.
  - Host: gate BN -> scalar affine (sg, Bg).
  - Pass 3: w = exp(silu(sg*gp + Bg)) (softmax w/o max-subtraction: BN-
    standardized gates are bounded, exp can't overflow); one-hot matmul
    scatter-accumulates [sum w*h, sum w] per 128-node group in PSUM; divide.
"""
import sys, os
sys.path.insert(0, '/opt/trn_rl_repo')
import numpy as np
import concourse.bass as bass
import concourse.bacc as bacc
import concourse.mybir as mybir
import concourse.tile as tile
from concourse.bass_utils import run_bass_kernel_spmd
from concourse.masks import make_identity

N, E, CIN, COUT, EPS = 100000, 1600000, 64, 64, 1e-5
NC = 8
NPC = 12544          # nodes per core (98 groups of 128)
NG = 98              # node groups per core
NCH, CHSZ = 4, 25600  # src chunks
BT = 16              # tiles per gather batch (2048 edges)
F32, BF16, I16 = mybir.dt.float32, mybir.dt.bfloat16, mybir.dt.int16
AF = mybir.ActivationFunctionType


def _prep(src, dst, pos_diff):
    """Returns global tile structure + per-core streams."""
    core = dst // NPC
    chunk = src // CHSZ
    grp = (dst - core * NPC) // 128
    key = (core * NCH + chunk) * NG + grp
    counts = np.bincount(key, minlength=NC * NCH * NG).reshape(NC, NCH, NG)
    T = -(-counts.max(axis=0) // 128)          # [NCH, NG] tiles per cell
    cells = [(c, g, int(T[c, g])) for c in range(NCH) for g in range(NG) if T[c, g] > 0]
    NT = sum(t for _, _, t in cells)
    # slot base per cell in (chunk, grp) order
    base = {}
    off = 0
    for c, g, t in cells:
        base[(c, g)] = off
        off += t * 128
    order = np.lexsort((grp, chunk, core))
    s_src, s_dst, s_core, s_chunk, s_grp = (a[order] for a in (src, dst, core, chunk, grp))
    s_pd = pos_diff[order]
    # rank within cell
    skey = (s_core * NCH + s_chunk) * NG + s_grp
    cell_start = np.zeros(NC * NCH * NG + 1, np.int64)
    np.cumsum(np.bincount(skey, minlength=NC * NCH * NG), out=cell_start[1:])
    rank = np.arange(len(s_src)) - cell_start[skey]
    base_arr = np.full(NC * NCH * NG, -1, np.int64)
    for c, g, t in cells:
        for co in range(NC):
            base_arr[(co * NCH + c) * NG + g] = base[(c, g)]
    slot = base_arr[skey] + rank

    # batches: per chunk, runs of <=BT tiles
    chunk_tiles = [sum(t for c, _, t in cells if c == ch) for ch in range(NCH)]
    batches = []  # (chunk, tile_off, ntiles)
    toff = 0
    for ch in range(NCH):
        rem = chunk_tiles[ch]
        while rem > 0:
            nb = min(BT, rem)
            batches.append((ch, toff, nb))
            toff += nb
            rem -= nb
    assert toff == NT

    per_core = []
    npad_chunk = np.zeros(NCH, np.int64)
    for co in range(NC):
        m = s_core == co
        sl, loc, dl = slot[m], (s_src[m] - s_chunk[m] * CHSZ), (s_dst[m] - co * NPC - s_grp[m] * 128)
        pd = s_pd[m]
        idx_flat = np.zeros(NT * 128, np.int32)
        dl_flat = np.full(NT * 128, 999.0, np.float32)
        hp_flat = np.zeros((NT * 128, 4), np.float32)
        idx_flat[sl] = loc
        dl_flat[sl] = dl
        hp_flat[sl, 0] = 1.0
        hp_flat[sl, 1:4] = pd
        # pad counts per chunk
        for c, g, t in cells:
            n_real = counts[co, c, g]
            npad_chunk[c] += t * 128 - n_real
        i16 = idx_flat.astype(np.int16)
        idx16 = np.tile(i16.reshape(NT * 8, 16).T, (8, 1))          # [128, NT*8]
        dlc = dl_flat.reshape(NT, 128).T.copy()                     # [128, NT]
        hpc = hp_flat.reshape(NT, 128, 4).transpose(1, 0, 2).reshape(128, NT * 4).copy()
        per_core.append(dict(idx16=np.ascontiguousarray(idx16), dl=dlc, hp=hpc))
    meta = dict(cells=cells, NT=NT, batches=batches, npad_chunk=npad_chunk)
    return meta, per_core


def _new_nc():
    return bacc.Bacc("TRN2", target_bir_lowering=False, debug=False, num_swdge_queues=4)


def _emit_gather(nc, pool, table_ap, idx_ap, gb, ch, toff, ntb, qn):
    ixt = pool.tile([128, ntb * 8], I16, tag="ixt")
    nc.sync.dma_start(out=ixt[:], in_=idx_ap[:, toff * 8: toff * 8 + ntb * 8])
    nc.gpsimd.dma_gather(
        out_ap=gb[:].rearrange("p (t d) -> p t d", d=64),
        in_ap=table_ap[ch * CHSZ: min((ch + 1) * CHSZ, N), :],
        idxs_ap=ixt[:],
        num_idxs=ntb * 128, num_idxs_reg=ntb * 128,
        elem_size=64, single_packet=False, queue_num=qn)


def build_pass1(meta):
    nc = _new_nc()
    table = nc.dram_tensor("table", [N, 64], F32, kind="ExternalInput").ap()
    NT = meta["NT"]
    idx = nc.dram_tensor("idx", [128, NT * 8], I16, kind="ExternalInput").ap()
    hp = nc.dram_tensor("hp", [128, NT * 4], F32, kind="ExternalInput").ap()
    gout = nc.dram_tensor("gout", [68, 68], F32, kind="ExternalOutput").ap()
    with tile.TileContext(nc) as tc:
        with tc.tile_pool(name="gb", bufs=3) as gbp, \
             tc.tile_pool(name="mt", bufs=4) as mtp, \
             tc.tile_pool(name="sm", bufs=2) as smp, \
             tc.tile_pool(name="ps", bufs=1, space="PSUM") as psp:
            G = psp.tile([68, 68], F32)
            t = 0
            for bi, (ch, toff, ntb) in enumerate(meta["batches"]):
                gb = gbp.tile([128, ntb * 64], F32, tag="gb")
                _emit_gather(nc, gbp, table, idx, gb, ch, toff, ntb, bi % 4)
                hpb = smp.tile([128, ntb * 4], F32, tag="hpb")
                nc.sync.dma_start(out=hpb[:], in_=hp[:, toff * 4: (toff + ntb) * 4])
                for j in range(ntb):
                    mt = mtp.tile([128, 68], F32, tag="mt")
                    nc.vector.tensor_copy(out=mt[:, 0:4], in_=hpb[:, j * 4:(j + 1) * 4])
                    nc.vector.tensor_copy(out=mt[:, 4:68], in_=gb[:, j * 64:(j + 1) * 64])
                    nc.tensor.matmul(out=G[:], lhsT=mt[:], rhs=mt[:],
                                     start=(t == 0), stop=(t == NT - 1))
                    t += 1
            gs = smp.tile([68, 68], F32, tag="gs")
            nc.vector.tensor_copy(out=gs[:], in_=G[:])
            nc.sync.dma_start(out=gout[:], in_=gs[:])
    nc.compile()
    return nc


def build_pass2(meta):
    nc = _new_nc()
    NT = meta["NT"]
    table = nc.dram_tensor("table", [N, 64], F32, kind="ExternalInput").ap()
    idx = nc.dram_tensor("idx", [128, NT * 8], I16, kind="ExternalInput").ap()
    hp = nc.dram_tensor("hp", [128, NT * 4], F32, kind="ExternalInput").ap()
    wp = nc.dram_tensor("wp", [68, 64], F32, kind="ExternalInput").ap()
    wg = nc.dram_tensor("wg", [128, 64], F32, kind="ExternalInput").ap()
    hdr = nc.dram_tensor("hdr", [128, NT * 64], BF16, kind="ExternalOutput").ap()
    gpd = nc.dram_tensor("gpd", [128, NT], F32, kind="ExternalOutput").ap()
    hgo = nc.dram_tensor("hgo", [65, 65], F32, kind="ExternalOutput").ap()
    with tile.TileContext(nc) as tc:
        with tc.tile_pool(name="gb", bufs=3) as gbp, \
             tc.tile_pool(name="mt", bufs=4) as mtp, \
             tc.tile_pool(name="sm", bufs=3) as smp, \
             tc.tile_pool(name="st", bufs=2) as stp, \
             tc.tile_pool(name="cn", bufs=1) as cnp, \
             tc.tile_pool(name="ps", bufs=1, space="PSUM") as psp, \
             tc.tile_pool(name="pt", bufs=3, space="PSUM") as ptp, \
             tc.tile_pool(name="ph", bufs=3, space="PSUM") as php:
            ident = cnp.tile([128, 128], F32)
            make_identity(nc, ident[:])
            wps = cnp.tile([68, 64], F32)
            nc.sync.dma_start(out=wps[:], in_=wp[:])
            wgs = cnp.tile([128, 64], F32)
            nc.sync.dma_start(out=wgs[:], in_=wg[:])
            HG = psp.tile([65, 65], F32)
            t = 0
            for bi, (ch, toff, ntb) in enumerate(meta["batches"]):
                gb = gbp.tile([128, ntb * 64], F32, tag="gb")
                _emit_gather(nc, gbp, table, idx, gb, ch, toff, ntb, bi % 4)
                hpb = smp.tile([128, ntb * 4], F32, tag="hpb")
                nc.sync.dma_start(out=hpb[:], in_=hp[:, toff * 4:(toff + ntb) * 4])
                hstg = stp.tile([128, ntb * 64], BF16, tag="hstg")
                gstg = stp.tile([128, ntb], F32, tag="gstg")
                for j in range(ntb):
                    mt = mtp.tile([128, 68], F32, tag="mt")
                    nc.vector.tensor_copy(out=mt[:, 0:4], in_=hpb[:, j * 4:(j + 1) * 4])
                    nc.vector.tensor_copy(out=mt[:, 4:68], in_=gb[:, j * 64:(j + 1) * 64])
                    pT = ptp.tile([68, 128], F32, tag="pT")
                    nc.tensor.transpose(out=pT[:], in_=mt[:], identity=ident[:])
                    mts = mtp.tile([68, 128], F32, tag="mts")
                    nc.vector.tensor_copy(out=mts[:], in_=pT[:])
                    hps = php.tile([128, 64], F32, tag="hps")
                    nc.tensor.matmul(out=hps[:], lhsT=mts[:], rhs=wps[:], start=True, stop=True)
                    h = mtp.tile([128, 65], F32, tag="h")
                    nc.scalar.activation(out=h[:, 0:64], in_=hps[:], func=AF.Silu)
                    nc.vector.memset(h[:, 64:65], 1.0)
                    nc.tensor.matmul(out=HG[:], lhsT=h[:], rhs=h[:],
                                     start=(t == 0), stop=(t == NT - 1))
                    tmp = mtp.tile([128, 64], F32, tag="tmp")
                    nc.vector.tensor_mul(out=tmp[:], in0=h[:, 0:64], in1=wgs[:])
                    nc.vector.reduce_sum(out=gstg[:, j:j + 1], in_=tmp[:], axis=mybir.AxisListType.X)
                    nc.vector.tensor_copy(out=hstg[:, j * 64:(j + 1) * 64], in_=h[:, 0:64])
                    t += 1
                nc.sync.dma_start(out=hdr[:, toff * 64:(toff + ntb) * 64], in_=hstg[:])
                nc.sync.dma_start(out=gpd[:, toff:toff + ntb], in_=gstg[:])
            hgs = smp.tile([65, 65], F32, tag="hgs")
            nc.vector.tensor_copy(out=hgs[:], in_=HG[:])
            nc.sync.dma_start(out=hgo[:], in_=hgs[:])
    nc.compile()
    return nc


def build_pass3(meta, sg, Bg):
    nc = _new_nc()
    NT = meta["NT"]
    hdr = nc.dram_tensor("hdr", [128, NT * 64], BF16, kind="ExternalInput").ap()
    gpd = nc.dram_tensor("gpd", [128, NT], F32, kind="ExternalInput").ap()
    dl = nc.dram_tensor("dl", [128, NT], F32, kind="ExternalInput").ap()
    iota = nc.dram_tensor("iota", [128, 128], F32, kind="ExternalInput").ap()
    out = nc.dram_tensor("out", [NPC, 64], F32, kind="ExternalOutput").ap()
    batches = meta["batches"]
    bat_of = {}
    for bi, (ch, toff, ntb) in enumerate(batches):
        for j in range(ntb):
            bat_of[toff + j] = (bi, j)
    with tile.TileContext(nc) as tc:
        with tc.tile_pool(name="hb", bufs=3) as hbp, \
             tc.tile_pool(name="sm", bufs=3) as smp, \
             tc.tile_pool(name="rh", bufs=4) as rhp, \
             tc.tile_pool(name="cn", bufs=1) as cnp, \
             tc.tile_pool(name="ac", bufs=1) as acp, \
             tc.tile_pool(name="pc", bufs=2, space="PSUM") as pcp:
            iot = cnp.tile([128, 128], F32)
            nc.sync.dma_start(out=iot[:], in_=iota[:])
            sgc = cnp.tile([128, 1], F32)
            nc.vector.memset(sgc[:], float(sg))
            bgc = cnp.tile([128, 1], F32)
            nc.vector.memset(bgc[:], float(Bg))
            acc = acp.tile([128, NG * 65], F32)
            nc.vector.memset(acc[:], 0.0)
            cur = [None, None, None, None, None][:4]  # hb, wbf, dlb tiles of current batch
            t = 0
            for (ch, g, tcnt) in meta["cells"]:
                ps = pcp.tile([128, 65], F32, tag="ps")
                for k in range(tcnt):
                    bi, j = bat_of[t]
                    if j == 0:
                        _, toff, ntb = batches[bi]
                        hb = hbp.tile([128, ntb * 64], BF16, tag="hb")
                        nc.sync.dma_start(out=hb[:], in_=hdr[:, toff * 64:(toff + ntb) * 64])
                        gpb = smp.tile([128, ntb], F32, tag="gpb")
                        nc.sync.dma_start(out=gpb[:], in_=gpd[:, toff:toff + ntb])
                        dlb = smp.tile([128, ntb], F32, tag="dlb")
                        nc.sync.dma_start(out=dlb[:], in_=dl[:, toff:toff + ntb])
                        sil = smp.tile([128, ntb], F32, tag="sil")
                        nc.scalar.activation(out=sil[:], in_=gpb[:], func=AF.Silu,
                                             scale=sgc[:], bias=bgc[:])
                        wb = smp.tile([128, ntb], F32, tag="wb")
                        nc.scalar.activation(out=wb[:], in_=sil[:], func=AF.Exp)
                        wbf = smp.tile([128, ntb], BF16, tag="wbf")
                        nc.vector.tensor_copy(out=wbf[:], in_=wb[:])
                        cur = [hb, wb, wbf, dlb]
                    hb, wb, wbf, dlb = cur
                    rhs = rhp.tile([128, 65], BF16, tag="rhs")
                    nc.vector.tensor_scalar(out=rhs[:, 0:64], in0=hb[:, j * 64:(j + 1) * 64],
                                            scalar1=wb[:, j:j + 1], scalar2=None,
                                            op0=mybir.AluOpType.mult)
                    nc.vector.tensor_copy(out=rhs[:, 64:65], in_=wbf[:, j:j + 1])
                    oh = rhp.tile([128, 128], BF16, tag="oh")
                    nc.vector.tensor_tensor(out=oh[:], in0=dlb[:, j:j + 1].to_broadcast([128, 128]),
                                            in1=iot[:], op=mybir.AluOpType.is_equal)
                    nc.tensor.matmul(out=ps[:], lhsT=oh[:], rhs=rhs[:],
                                     start=(k == 0), stop=(k == tcnt - 1))
                    t += 1
                nc.vector.tensor_add(out=acc[:, g * 65:(g + 1) * 65],
                                     in0=acc[:, g * 65:(g + 1) * 65], in1=ps[:])
            obuf = acp.tile([128, NG * 64], F32)
            for g in range(NG):
                dmx = smp.tile([128, 1], F32, tag="dmx")
                nc.vector.tensor_scalar_max(out=dmx[:], in0=acc[:, g * 65 + 64:g * 65 + 65], scalar1=1e-16)
                rec = smp.tile([128, 1], F32, tag="rec")
                nc.vector.reciprocal(out=rec[:], in_=dmx[:])
                nc.vector.tensor_scalar(out=obuf[:, g * 64:(g + 1) * 64],
                                        in0=acc[:, g * 65:g * 65 + 64],
                                        scalar1=rec[:], scalar2=None, op0=mybir.AluOpType.mult)
            nc.sync.dma_start(out=out.rearrange("(g p) d -> p g d", p=128), in_=obuf[:].rearrange("p (g d) -> p g d", d=64))
    nc.compile()
    return nc


def kernel(x, pos, W1, b1, bn1_g, bn1_b, Wg, bg, bng_g, bng_b, edge_index):
    x = np.asarray(x, np.float32)
    pos = np.asarray(pos, np.float32)
    ei = np.asarray(edge_index)
    src = ei[0].astype(np.int64)
    dst = ei[1].astype(np.int64)
    W1, b1 = np.asarray(W1, np.float64), np.asarray(b1, np.float64)
    bn1_g, bn1_b = np.asarray(bn1_g, np.float64), np.asarray(bn1_b, np.float64)
    Wg, bg = np.asarray(Wg, np.float64), float(np.asarray(bg).ravel()[0])
    bng_g, bng_b = float(np.asarray(bng_g).ravel()[0]), float(np.asarray(bng_b).ravel()[0])

    pos_diff = (pos[src] - pos[dst]).astype(np.float32)
    meta, per_core = _prep(src, dst, pos_diff)
    cores = list(range(NC))

    # ---- pass 1: Gram of m~ ----
    nc1 = build_pass1(meta)
    in1 = [dict(table=x, idx=pc["idx16"], hp=pc["hp"]) for pc in per_core]
    r1 = run_bass_kernel_spmd(nc1, in1, cores)
    G = np.zeros((68, 68), np.float64)
    for c in range(NC):
        G += r1.results[c]["gout"].astype(np.float64)
    for ch in range(NCH):
        npad = meta["npad_chunk"][ch]
        if npad:
            v = np.zeros(68); v[4:68] = x[ch * CHSZ].astype(np.float64)
            G -= npad * np.outer(v, v)
    # W~1 rows: [b1, Wpos(3), Wx(64)]
    Wt = np.zeros((68, 64)); Wt[0] = b1; Wt[1:4] = W1[0:3]; Wt[4:68] = W1[3:67]
    sum_h = Wt.T @ G[:, 0]
    sumsq = np.einsum('kc,kl,lc->c', Wt, G, Wt)
    mu = sum_h / E
    var = sumsq / E - mu * mu
    s = bn1_g / np.sqrt(var + EPS)
    tsh = bn1_b - mu * s
    Wp = (Wt * s[None, :]); Wp[0] += tsh
    Wp = Wp.astype(np.float32)

    # ---- pass 2: h + gate-pre + h~ Gram ----
    nc2 = build_pass2(meta)
    wg_rep = np.tile(Wg.ravel().astype(np.float32)[None, :], (128, 1))
    in2 = [dict(table=x, idx=pc["idx16"], hp=pc["hp"], wp=Wp, wg=wg_rep) for pc in per_core]
    r2 = run_bass_kernel_spmd(nc2, in2, cores)
    HG = np.zeros((65, 65), np.float64)
    for c in range(NC):
        HG += r2.results[c]["hgo"].astype(np.float64)
    for ch in range(NCH):
        npad = meta["npad_chunk"][ch]
        if npad:
            mpad = np.zeros(68); mpad[4:68] = x[ch * CHSZ].astype(np.float64)
            hpre = mpad @ Wp.astype(np.float64)
            hpad = hpre / (1 + np.exp(-hpre))
            hv = np.concatenate([hpad, [1.0]])
            HG -= npad * np.outer(hv, hv)
    wgv = Wg.ravel()
    sum_gp = wgv @ HG[0:64, 64]
    sumsq_gp = wgv @ HG[0:64, 0:64] @ wgv
    mu_g = sum_gp / E + bg
    var_g = sumsq_gp / E - (sum_gp / E) ** 2
    sg = bng_g / np.sqrt(var_g + EPS)
    Bg = bng_b + sg * (bg - mu_g)

    # ---- pass 3: softmax-weighted scatter ----
    nc3 = build_pass3(meta, sg, Bg)
    iota_np = np.tile(np.arange(128, dtype=np.float32)[None, :], (128, 1))
    in3 = []
    for c in range(NC):
        in3.append(dict(hdr=r2.results[c]["hdr"], gpd=r2.results[c]["gpd"],
                        dl=per_core[c]["dl"], iota=iota_np))
    r3 = run_bass_kernel_spmd(nc3, in3, cores)
    out = np.zeros((N, 64), np.float32)
    for c in range(NC):
        lo = c * NPC
        hi = min(lo + NPC, N)
        out[lo:hi] = r3.results[c]["out"][0:hi - lo]
    return out


# revision 10
# speedup vs baseline: 1.3064x; 1.3064x over previous
"""AttnPointNetConv on 8 TRN2 NeuronCores.

Strategy (edge-parallel, dst-sharded):
  - Host: sort edges by (dst-core, src-chunk, dst-group). Core c owns nodes
    [c*12544, (c+1)*12544). src space split into 4 chunks of 25600 so
    dma_gather's int16 indices can address the x-table chunk.
  - Pass 1: per-edge gather of x[src]; PE accumulates the Gram matrix
    G = sum(m~ m~^T) of the augmented message m~ = [1, pos_j-pos_i, x_j].
    Host derives BN1 stats from G (linear algebra) and folds BN1 into the
    MLP weight W' (affine fold), incl. exact removal of padding edges.
  - Pass 2: recompute m~, h = silu(m~ @ W'), PE accumulates h~-Gram for the
    gate BN stats; saves h (bf16) and gate_pre (f32) to DRAM.
  - Host: gate BN -> scalar affine (sg, Bg).
  - Pass 3: w = exp(silu(sg*gp + Bg)) (softmax w/o max-subtraction: BN-
    standardized gates are bounded, exp can't overflow); one-hot matmul
    scatter-accumulates [sum w*h, sum w] per 128-node group in PSUM; divide.
"""
import sys, os
sys.path.insert(0, '/opt/trn_rl_repo')
import numpy as np
import concourse.bass as bass
import concourse.bacc as bacc
import concourse.mybir as mybir
import concourse.tile as tile
from concourse.bass_utils import run_bass_kernel_spmd
from concourse.masks import make_identity

N, E, CIN, COUT, EPS = 100000, 1600000, 64, 64, 1e-5
NPAD = 100352        # x table padded (8*12544) for uniform per-core slices
NC = 8
NPC = 12544          # nodes per core (98 groups of 128)
NG = 98              # node groups per core
NCH, CHSZ = 4, 25600  # src chunks
BT = 64              # tiles per gather batch (8192 edges)
F32, BF16, I16 = mybir.dt.float32, mybir.dt.bfloat16, mybir.dt.int16
AF = mybir.ActivationFunctionType


def _prep(src, dst, pos_diff):
    """Returns global tile structure + per-core streams."""
    core = dst // NPC
    chunk = src // CHSZ
    grp = (dst - core * NPC) // 128
    key = (core * NCH + chunk) * NG + grp
    counts = np.bincount(key, minlength=NC * NCH * NG).reshape(NC, NCH, NG)
    T = -(-counts.max(axis=0) // 128)          # [NCH, NG] tiles per cell
    cells = [(c, g, int(T[c, g])) for c in range(NCH) for g in range(NG) if T[c, g] > 0]
    NT = sum(t for _, _, t in cells)
    # slot base per cell in (chunk, grp) order
    base = {}
    off = 0
    for c, g, t in cells:
        base[(c, g)] = off
        off += t * 128
    order = np.lexsort((grp, chunk, core))
    s_src, s_dst, s_core, s_chunk, s_grp = (a[order] for a in (src, dst, core, chunk, grp))
    s_pd = pos_diff[order]
    # rank within cell
    skey = (s_core * NCH + s_chunk) * NG + s_grp
    cell_start = np.zeros(NC * NCH * NG + 1, np.int64)
    np.cumsum(np.bincount(skey, minlength=NC * NCH * NG), out=cell_start[1:])
    rank = np.arange(len(s_src)) - cell_start[skey]
    base_arr = np.full(NC * NCH * NG, -1, np.int64)
    for c, g, t in cells:
        for co in range(NC):
            base_arr[(co * NCH + c) * NG + g] = base[(c, g)]
    slot = base_arr[skey] + rank

    # batches: per chunk, runs of <=BT tiles
    chunk_tiles = [sum(t for c, _, t in cells if c == ch) for ch in range(NCH)]
    batches = []  # (chunk, tile_off, ntiles)
    toff = 0
    for ch in range(NCH):
        rem = chunk_tiles[ch]
        while rem > 0:
            nb = min(BT, rem)
            batches.append((ch, toff, nb))
            toff += nb
            rem -= nb
    assert toff == NT

    per_core = []
    npad_chunk = np.zeros(NCH, np.int64)
    for co in range(NC):
        m = s_core == co
        sl, loc, dl = slot[m], (s_src[m] - s_chunk[m] * CHSZ), (s_dst[m] - co * NPC - s_grp[m] * 128)
        pd = s_pd[m]
        idx_flat = np.zeros(NT * 128, np.int32)
        dl_flat = np.full(NT * 128, 999.0, np.float32)
        hp_flat = np.zeros((NT * 128, 4), np.float32)
        idx_flat[sl] = loc
        dl_flat[sl] = dl
        hp_flat[sl, 0] = 1.0
        hp_flat[sl, 1:4] = pd
        # pad counts per chunk
        for c, g, t in cells:
            n_real = counts[co, c, g]
            npad_chunk[c] += t * 128 - n_real
        i16 = idx_flat.astype(np.int16)
        idx16 = np.tile(i16.reshape(NT * 8, 16).T, (8, 1))          # [128, NT*8]
        dlc = dl_flat.reshape(NT, 128).T.copy()                     # [128, NT]
        hpc = hp_flat.reshape(NT, 128, 4).transpose(1, 0, 2).reshape(128, NT * 4).copy()
        per_core.append(dict(idx16=np.ascontiguousarray(idx16), dl=dlc, hp=hpc))
    meta = dict(cells=cells, NT=NT, batches=batches, npad_chunk=npad_chunk)
    return meta, per_core


def _new_nc():
    return bacc.Bacc("TRN2", target_bir_lowering=False, debug=False, num_swdge_queues=4)


def _emit_gather(nc, pool, table_ap, idx_ap, gb, ch, toff, ntb, qn):
    ixt = pool.tile([128, ntb * 8], I16, tag="ixt")
    nc.sync.dma_start(out=ixt[:], in_=idx_ap[:, toff * 8: toff * 8 + ntb * 8])
    nc.gpsimd.dma_gather(
        out_ap=gb[:].rearrange("p (t d) -> p t d", d=64),
        in_ap=table_ap[ch * CHSZ: min((ch + 1) * CHSZ, NPAD), :],
        idxs_ap=ixt[:],
        num_idxs=ntb * 128, num_idxs_reg=ntb * 128,
        elem_size=64, single_packet=False, queue_num=qn)


def build_pass1(meta):
    """Node-streaming stats: Gxx = sum_j deg_j x_j x_j^T via sqrt-deg weighting,
    Gax = sum_j A_j x_j^T. 98 sequential node tiles per core; no gather."""
    nc = _new_nc()
    xs = nc.dram_tensor("xs", [128, 98 * 64], F32, kind="ExternalInput").ap()
    sqd = nc.dram_tensor("sqd", [128, 98], F32, kind="ExternalInput").ap()
    aarr = nc.dram_tensor("aarr", [128, 98 * 4], F32, kind="ExternalInput").ap()
    gxxo = nc.dram_tensor("gxxo", [64, 64], F32, kind="ExternalOutput").ap()
    gaxo = nc.dram_tensor("gaxo", [4, 64], F32, kind="ExternalOutput").ap()
    with tile.TileContext(nc) as tc:
        with tc.tile_pool(name="sb", bufs=4) as sbp, \
             tc.tile_pool(name="sm", bufs=2) as smp, \
             tc.tile_pool(name="ps", bufs=1, space="PSUM") as psp:
            sq = smp.tile([128, 98], F32, tag="sq")
            nc.sync.dma_start(out=sq[:], in_=sqd[:])
            Gxx = psp.tile([64, 64], F32)
            Gax = psp.tile([4, 64], F32)
            for t in range(98):
                xt = sbp.tile([128, 64], F32, tag="xt")
                nc.sync.dma_start(out=xt[:], in_=xs[:, t * 64:(t + 1) * 64])
                at = sbp.tile([128, 4], F32, tag="at")
                nc.sync.dma_start(out=at[:], in_=aarr[:, t * 4:(t + 1) * 4])
                xw = sbp.tile([128, 64], F32, tag="xw")
                nc.vector.tensor_scalar(out=xw[:], in0=xt[:], scalar1=sq[:, t:t + 1],
                                        scalar2=None, op0=mybir.AluOpType.mult)
                nc.tensor.matmul(out=Gxx[:], lhsT=xw[:], rhs=xw[:],
                                 start=(t == 0), stop=(t == 97))
                nc.tensor.matmul(out=Gax[:], lhsT=at[:], rhs=xt[:],
                                 start=(t == 0), stop=(t == 97))
            gs = smp.tile([64, 64], F32, tag="gs")
            nc.vector.tensor_copy(out=gs[:], in_=Gxx[:])
            nc.sync.dma_start(out=gxxo[:], in_=gs[:])
            ga = smp.tile([4, 64], F32, tag="ga")
            nc.vector.tensor_copy(out=ga[:], in_=Gax[:])
            nc.sync.dma_start(out=gaxo[:], in_=ga[:])
    nc.compile()
    return nc


def build_pass2(meta):
    nc = _new_nc()
    NT = meta["NT"]
    table = nc.dram_tensor("table", [NPAD, 64], F32, kind="ExternalInput").ap()
    idx = nc.dram_tensor("idx", [128, NT * 8], I16, kind="ExternalInput").ap()
    hp = nc.dram_tensor("hp", [128, NT * 4], F32, kind="ExternalInput").ap()
    wp = nc.dram_tensor("wp", [68, 64], F32, kind="ExternalInput").ap()
    wg = nc.dram_tensor("wg", [128, 64], F32, kind="ExternalInput").ap()
    hdr = nc.dram_tensor("hdr", [128, NT * 64], BF16, kind="ExternalOutput").ap()
    gpd = nc.dram_tensor("gpd", [128, NT], F32, kind="ExternalOutput").ap()
    hgo = nc.dram_tensor("hgo", [65, 65], F32, kind="ExternalOutput").ap()
    with tile.TileContext(nc) as tc:
        with tc.tile_pool(name="gb", bufs=3) as gbp, \
             tc.tile_pool(name="mt", bufs=4) as mtp, \
             tc.tile_pool(name="sm", bufs=3) as smp, \
             tc.tile_pool(name="st", bufs=2) as stp, \
             tc.tile_pool(name="cn", bufs=1) as cnp, \
             tc.tile_pool(name="ps", bufs=1, space="PSUM") as psp, \
             tc.tile_pool(name="pt", bufs=3, space="PSUM") as ptp, \
             tc.tile_pool(name="ph", bufs=3, space="PSUM") as php:
            ident = cnp.tile([128, 128], F32)
            make_identity(nc, ident[:])
            wps = cnp.tile([68, 64], F32)
            nc.sync.dma_start(out=wps[:], in_=wp[:])
            wgs = cnp.tile([128, 64], F32)
            nc.sync.dma_start(out=wgs[:], in_=wg[:])
            HG = psp.tile([65, 65], F32)
            t = 0
            for bi, (ch, toff, ntb) in enumerate(meta["batches"]):
                gb = gbp.tile([128, ntb * 64], F32, tag="gb")
                _emit_gather(nc, gbp, table, idx, gb, ch, toff, ntb, bi % 4)
                hpb = smp.tile([128, ntb * 4], F32, tag="hpb")
                nc.sync.dma_start(out=hpb[:], in_=hp[:, toff * 4:(toff + ntb) * 4])
                hstg = stp.tile([128, ntb * 64], BF16, tag="hstg")
                gstg = stp.tile([128, ntb], F32, tag="gstg")
                for j in range(ntb):
                    mt = mtp.tile([128, 68], F32, tag="mt")
                    nc.vector.tensor_copy(out=mt[:, 0:4], in_=hpb[:, j * 4:(j + 1) * 4])
                    nc.vector.tensor_copy(out=mt[:, 4:68], in_=gb[:, j * 64:(j + 1) * 64])
                    pT = ptp.tile([68, 128], F32, tag="pT")
                    nc.tensor.transpose(out=pT[:], in_=mt[:], identity=ident[:])
                    mts = mtp.tile([68, 128], F32, tag="mts")
                    nc.vector.tensor_copy(out=mts[:], in_=pT[:])
                    hps = php.tile([128, 64], F32, tag="hps")
                    nc.tensor.matmul(out=hps[:], lhsT=mts[:], rhs=wps[:], start=True, stop=True)
                    h = mtp.tile([128, 65], F32, tag="h")
                    nc.scalar.activation(out=h[:, 0:64], in_=hps[:], func=AF.Silu)
                    nc.vector.memset(h[:, 64:65], 1.0)
                    nc.tensor.matmul(out=HG[:], lhsT=h[:], rhs=h[:],
                                     start=(t == 0), stop=(t == NT - 1))
                    tmp = mtp.tile([128, 64], F32, tag="tmp")
                    nc.vector.tensor_mul(out=tmp[:], in0=h[:, 0:64], in1=wgs[:])
                    nc.vector.reduce_sum(out=gstg[:, j:j + 1], in_=tmp[:], axis=mybir.AxisListType.X)
                    nc.vector.tensor_copy(out=hstg[:, j * 64:(j + 1) * 64], in_=h[:, 0:64])
                    t += 1
                nc.sync.dma_start(out=hdr[:, toff * 64:(toff + ntb) * 64], in_=hstg[:])
                nc.sync.dma_start(out=gpd[:, toff:toff + ntb], in_=gstg[:])
            hgs = smp.tile([65, 65], F32, tag="hgs")
            nc.vector.tensor_copy(out=hgs[:], in_=HG[:])
            nc.sync.dma_start(out=hgo[:], in_=hgs[:])
    nc.compile()
    return nc


def build_pass3(meta, sg, Bg):
    nc = _new_nc()
    NT = meta["NT"]
    hdr = nc.dram_tensor("hdr", [128, NT * 64], BF16, kind="ExternalInput").ap()
    gpd = nc.dram_tensor("gpd", [128, NT], F32, kind="ExternalInput").ap()
    dl = nc.dram_tensor("dl", [128, NT], F32, kind="ExternalInput").ap()
    iota = nc.dram_tensor("iota", [128, 128], F32, kind="ExternalInput").ap()
    out = nc.dram_tensor("out", [NPC, 64], F32, kind="ExternalOutput").ap()
    batches = meta["batches"]
    bat_of = {}
    for bi, (ch, toff, ntb) in enumerate(batches):
        for j in range(ntb):
            bat_of[toff + j] = (bi, j)
    with tile.TileContext(nc) as tc:
        with tc.tile_pool(name="hb", bufs=3) as hbp, \
             tc.tile_pool(name="sm", bufs=3) as smp, \
             tc.tile_pool(name="rh", bufs=4) as rhp, \
             tc.tile_pool(name="cn", bufs=1) as cnp, \
             tc.tile_pool(name="ac", bufs=1) as acp, \
             tc.tile_pool(name="pc", bufs=2, space="PSUM") as pcp:
            iot = cnp.tile([128, 128], F32)
            nc.sync.dma_start(out=iot[:], in_=iota[:])
            sgc = cnp.tile([128, 1], F32)
            nc.vector.memset(sgc[:], float(sg))
            bgc = cnp.tile([128, 1], F32)
            nc.vector.memset(bgc[:], float(Bg))
            acc = acp.tile([128, NG * 65], F32)
            nc.vector.memset(acc[:], 0.0)
            cur = [None, None, None, None, None][:4]  # hb, wbf, dlb tiles of current batch
            t = 0
            for (ch, g, tcnt) in meta["cells"]:
                ps = pcp.tile([128, 65], F32, tag="ps")
                for k in range(tcnt):
                    bi, j = bat_of[t]
                    if j == 0:
                        _, toff, ntb = batches[bi]
                        hb = hbp.tile([128, ntb * 64], BF16, tag="hb")
                        nc.sync.dma_start(out=hb[:], in_=hdr[:, toff * 64:(toff + ntb) * 64])
                        gpb = smp.tile([128, ntb], F32, tag="gpb")
                        nc.sync.dma_start(out=gpb[:], in_=gpd[:, toff:toff + ntb])
                        dlb = smp.tile([128, ntb], F32, tag="dlb")
                        nc.sync.dma_start(out=dlb[:], in_=dl[:, toff:toff + ntb])
                        sil = smp.tile([128, ntb], F32, tag="sil")
                        nc.scalar.activation(out=sil[:], in_=gpb[:], func=AF.Silu,
                                             scale=sgc[:], bias=bgc[:])
                        wb = smp.tile([128, ntb], F32, tag="wb")
                        nc.scalar.activation(out=wb[:], in_=sil[:], func=AF.Exp)
                        wbf = smp.tile([128, ntb], BF16, tag="wbf")
                        nc.vector.tensor_copy(out=wbf[:], in_=wb[:])
                        cur = [hb, wb, wbf, dlb]
                    hb, wb, wbf, dlb = cur
                    rhs = rhp.tile([128, 65], BF16, tag="rhs")
                    nc.vector.tensor_scalar(out=rhs[:, 0:64], in0=hb[:, j * 64:(j + 1) * 64],
                                            scalar1=wb[:, j:j + 1], scalar2=None,
                                            op0=mybir.AluOpType.mult)
                    nc.vector.tensor_copy(out=rhs[:, 64:65], in_=wbf[:, j:j + 1])
                    oh = rhp.tile([128, 128], BF16, tag="oh")
                    nc.vector.tensor_tensor(out=oh[:], in0=dlb[:, j:j + 1].to_broadcast([128, 128]),
                                            in1=iot[:], op=mybir.AluOpType.is_equal)
                    nc.tensor.matmul(out=ps[:], lhsT=oh[:], rhs=rhs[:],
                                     start=(k == 0), stop=(k == tcnt - 1))
                    t += 1
                nc.vector.tensor_add(out=acc[:, g * 65:(g + 1) * 65],
                                     in0=acc[:, g * 65:(g + 1) * 65], in1=ps[:])
            obuf = acp.tile([128, NG * 64], F32)
            for g in range(NG):
                dmx = smp.tile([128, 1], F32, tag="dmx")
                nc.vector.tensor_scalar_max(out=dmx[:], in0=acc[:, g * 65 + 64:g * 65 + 65], scalar1=1e-16)
                rec = smp.tile([128, 1], F32, tag="rec")
                nc.vector.reciprocal(out=rec[:], in_=dmx[:])
                nc.vector.tensor_scalar(out=obuf[:, g * 64:(g + 1) * 64],
                                        in0=acc[:, g * 65:g * 65 + 64],
                                        scalar1=rec[:], scalar2=None, op0=mybir.AluOpType.mult)
            nc.sync.dma_start(out=out.rearrange("(g p) d -> p g d", p=128), in_=obuf[:].rearrange("p (g d) -> p g d", d=64))
    nc.compile()
    return nc


def kernel(x, pos, W1, b1, bn1_g, bn1_b, Wg, bg, bng_g, bng_b, edge_index):
    x = np.asarray(x, np.float32)
    pos = np.asarray(pos, np.float32)
    ei = np.asarray(edge_index)
    src = ei[0].astype(np.int64)
    dst = ei[1].astype(np.int64)
    W1, b1 = np.asarray(W1, np.float64), np.asarray(b1, np.float64)
    bn1_g, bn1_b = np.asarray(bn1_g, np.float64), np.asarray(bn1_b, np.float64)
    Wg, bg = np.asarray(Wg, np.float64), float(np.asarray(bg).ravel()[0])
    bng_g, bng_b = float(np.asarray(bng_g).ravel()[0]), float(np.asarray(bng_b).ravel()[0])

    pos_diff = (pos[src] - pos[dst]).astype(np.float32)
    meta, per_core = _prep(src, dst, pos_diff)
    cores = list(range(NC))
    xpad = np.zeros((NPAD, 64), np.float32)
    xpad[:N] = x

    # ---- pass 1: node-streaming Gram for BN1 stats ----
    deg = np.bincount(src, minlength=NPAD).astype(np.float64)
    A = np.zeros((NPAD, 4), np.float64)
    A[:, 0] = deg
    for k in range(3):
        A[:, 1 + k] = np.bincount(src, weights=pos_diff[:, k].astype(np.float64), minlength=NPAD)
    sqd_full = np.sqrt(deg).astype(np.float32)
    nc1 = build_pass1(meta)
    in1 = []
    for c in range(NC):
        lo = c * NPC
        in1.append(dict(
            xs=xpad[lo:lo + NPC].reshape(98, 128, 64).transpose(1, 0, 2).reshape(128, 98 * 64).copy(),
            sqd=sqd_full[lo:lo + NPC].reshape(98, 128).T.copy(),
            aarr=A[lo:lo + NPC].astype(np.float32).reshape(98, 128, 4).transpose(1, 0, 2).reshape(128, 98 * 4).copy()))
    r1 = run_bass_kernel_spmd(nc1, in1, cores)
    G = np.zeros((68, 68), np.float64)
    for c in range(NC):
        G[4:68, 4:68] += r1.results[c]["gxxo"].astype(np.float64)
        G[0:4, 4:68] += r1.results[c]["gaxo"].astype(np.float64)
    G[4:68, 0:4] = G[0:4, 4:68].T
    # [0:4, 0:4] block: direct host stats over edges
    G[0, 0] = E
    G[0, 1:4] = pos_diff.astype(np.float64).sum(axis=0)
    G[1:4, 0] = G[0, 1:4]
    G[1:4, 1:4] = pos_diff.astype(np.float64).T @ pos_diff.astype(np.float64)
    # W~1 rows: [b1, Wpos(3), Wx(64)]
    Wt = np.zeros((68, 64)); Wt[0] = b1; Wt[1:4] = W1[0:3]; Wt[4:68] = W1[3:67]
    sum_h = Wt.T @ G[:, 0]
    sumsq = np.einsum('kc,kl,lc->c', Wt, G, Wt)
    mu = sum_h / E
    var = sumsq / E - mu * mu
    s = bn1_g / np.sqrt(var + EPS)
    tsh = bn1_b - mu * s
    Wp = (Wt * s[None, :]); Wp[0] += tsh
    Wp = Wp.astype(np.float32)
    # ---- pass 2: h + gate-pre + h~ Gram ----
    nc2 = build_pass2(meta)
    wg_rep = np.tile(Wg.ravel().astype(np.float32)[None, :], (128, 1))
    in2 = [dict(table=xpad, idx=pc["idx16"], hp=pc["hp"], wp=Wp, wg=wg_rep) for pc in per_core]
    r2 = run_bass_kernel_spmd(nc2, in2, cores)
    HG = np.zeros((65, 65), np.float64)
    for c in range(NC):
        HG += r2.results[c]["hgo"].astype(np.float64)
    for ch in range(NCH):
        npad = meta["npad_chunk"][ch]
        if npad:
            mpad = np.zeros(68); mpad[4:68] = x[ch * CHSZ].astype(np.float64)
            hpre = mpad @ Wp.astype(np.float64)
            hpad = hpre / (1 + np.exp(-hpre))
            hv = np.concatenate([hpad, [1.0]])
            HG -= npad * np.outer(hv, hv)
    wgv = Wg.ravel()
    sum_gp = wgv @ HG[0:64, 64]
    sumsq_gp = wgv @ HG[0:64, 0:64] @ wgv
    mu_g = sum_gp / E + bg
    var_g = sumsq_gp / E - (sum_gp / E) ** 2
    sg = bng_g / np.sqrt(var_g + EPS)
    Bg = bng_b + sg * (bg - mu_g)

    # ---- pass 3: softmax-weighted scatter ----
    nc3 = build_pass3(meta, sg, Bg)
    iota_np = np.tile(np.arange(128, dtype=np.float32)[None, :], (128, 1))
    in3 = []
    for c in range(NC):
        in3.append(dict(hdr=r2.results[c]["hdr"], gpd=r2.results[c]["gpd"],
                        dl=per_core[c]["dl"], iota=iota_np))
    r3 = run_bass_kernel_spmd(nc3, in3, cores)
    out = np.zeros((N, 64), np.float32)
    for c in range(NC):
        lo = c * NPC
        hi = min(lo + NPC, N)
        out[lo:hi] = r3.results[c]["out"][0:hi - lo]
    return out
